# revision 31
# baseline (speedup 1.0000x reference)
"""MHA kernel for TRN2: B=4,T=2048,D=1024,H=16,HD=64 across 8 NeuronCores.

Sharding: core c -> batch c//2, query half c%2 (host rotates the sequence so
each core's queries are rows 0:1024; softmax over keys is permutation
invariant). No collectives.

Design (v3 -- reoriented PV + cycle-paced fillers):
- bf16 everywhere; x^T and W^T are pre-transposed/cast/regrouped on the
  host so every weight slice arrives in one large DMA.
- Logits per (head, s-chunk): out [128 s, 2x512 q] from K-stationary
  matmuls (contraction hd=64 via base-partition 64/0 tiles).
- exp on ScalarE over [128, 1024] tiles -> pt ring (32 slots) in SBUF.
- PV is REORIENTED: probs are the stationary operand [128 s, 128 q],
  V (+ones col) the moving [128 s, 65] -> out [128 q, 65] accumulated over
  all of s into a single PSUM bank per q-chunk (qc-outer sweep, one open
  accumulation group at a time, swept during the next head's step).
  This runs PV at full PE efficiency: 262k -> 133k PE cycles.
- Per-q denominators land in pv[:, 64] (ones column); normalization is a
  DVE reciprocal_approx + per-partition-scalar multiply on GpSimd (q is on
  partitions now).
- cat comes out [q, hd]; per group of 2 heads it is transposed back to
  [hd2, q] with 8 PE-transpose ops into one PSUM bank, then one copy.
- V is computed per head-pair just-in-time (vt pool of 3 pair tiles keeps
  SBUF small enough for the 32-slot pt ring); Wo reuses Wv's SBUF slot.
- Projection/V work is queued as ~1k/4k-cycle filler units at WAR-safe
  points and popped under a cycle-paced budget (~1.3k PE cycles per sc
  step) so TensorE stays saturated through the whole ACT-bound midgame;
  per-unit PSUM work tiles are allocated at emission time so pool rotation
  order matches the instruction stream.
"""
import sys
sys.path.insert(0, "/opt/trn_rl_repo")
import warnings
warnings.filterwarnings("ignore")

import numpy as np
import ml_dtypes
import concourse.bass as bass
import concourse.mybir as mybir
import concourse.tile as tile
from concourse import bacc
from concourse.bass_utils import run_bass_kernel_spmd

F32 = mybir.dt.float32
BF = mybir.dt.bfloat16
EXP = mybir.ActivationFunctionType.Exp
MUL = mybir.AluOpType.mult

T, D = 2048, 1024
TQ = 1024          # queries per core
NH = 16            # heads
NSC = 16           # s chunks of 128
NDC = 8            # d chunks of 128
NQC = 8            # q chunks of 128
SCALE = 0.125      # 1/sqrt(64)

V_COST, P_COST = 1024, 4096    # PE cycles per filler unit
PACE_A, PACE_B = 1250, 900     # filler pace (cycles/sc): first/second half

# Schraudolph exp on DVE for a few s-chunks per head: exp(s*x) ~=
# bitcast_f32(int32(x*A + B)); emitting uint16(x*A/2^16 + B/2^16) yields
# the TOP half of that fp32 word, i.e. the bf16 bit pattern, directly into
# the regular bf16 pt ring. ~1.5% rms prob error on those chunks, which
# the softmax tolerates (measured end-to-end ~4e-3 rel).
SCH_A = float(1 << 23) * 1.4426950408889634 * SCALE / 65536.0
SCH_B = 128.0 * (127.0 - 0.0440)
U16 = mybir.dt.uint16
ADD = mybir.AluOpType.add


RG = [[0, 1], [2, 3], [4, 5], [6, 7]]   # HBM-pair replica groups
BYP = mybir.AluOpType.bypass


def dve_sc(h):
    # late steps run filler-dry (split K/V work lives early), so shift an
    # increasing share of the exp load off ACT onto DVE there
    if h >= NH - 3:
        return (1, 3, 5, 7, 9, 11, 13, 15)
    if h >= NH - 6:
        return (1, 5, 9, 13)
    return (5, 13)


def build_nc():
    nc = bacc.Bacc("TRN2", target_bir_lowering=False, debug=False, num_devices=8)
    # xt: [4 tblk, 128, NDC, 512]; wq/wk/wv: [8 colblk, 128, NDC, 128]
    xt = nc.dram_tensor("xt", [4, 128, NDC, 512], BF, kind="ExternalInput")
    wqt = nc.dram_tensor("wqt", [8, 128, NDC, 128], BF, kind="ExternalInput")
    wkt = nc.dram_tensor("wkt", [8, 128, NDC, 128], BF, kind="ExternalInput")
    wvt = nc.dram_tensor("wvt", [8, 128, NDC, 128], BF, kind="ExternalInput")
    wot = nc.dram_tensor("wot", [NDC, 128, D], BF, kind="ExternalInput")
    bo = nc.dram_tensor("bo", [1, D], BF, kind="ExternalInput")
    idt = nc.dram_tensor("idt", [128, 128], BF, kind="ExternalInput")
    y = nc.dram_tensor("y", [TQ, D], F32, kind="ExternalOutput")
    # staging for the pairwise K/V AllGathers (each core computes its local
    # half of K and V -- rows 0:1024 of its own rotated sequence -- and the
    # HBM-pair partner supplies the other half; gather rank order gives both
    # cores the same key order, and softmax+PV are key-permutation invariant)
    kso = nc.dram_tensor("kso", [8, 128, TQ], BF, kind="Internal")
    ksi = nc.dram_tensor("ksi", [16, 128, TQ], BF, kind="Internal")
    vso = nc.dram_tensor("vso", [8, 128, 1040], BF, kind="Internal")
    vsi = nc.dram_tensor("vsi", [16, 128, 1040], BF, kind="Internal")

    with tile.TileContext(nc) as tc:
        with (
            tc.tile_pool(name="persist", bufs=1) as pp,
            tc.tile_pool(name="wsh", bufs=1) as wsh,
            tc.tile_pool(name="vtp", bufs=3) as vtp,
            tc.tile_pool(name="qk", bufs=3) as qk,
            tc.tile_pool(name="ptp", bufs=32) as ptp,
            tc.tile_pool(name="cq", bufs=2) as cq,
            tc.tile_pool(name="rcp", bufs=8) as rcp,
            tc.tile_pool(name="yp", bufs=2) as yp,
            tc.tile_pool(name="ps_lg", bufs=2, space="PSUM") as pslg,
            tc.tile_pool(name="ps_pv", bufs=2, space="PSUM") as pspv,
            tc.tile_pool(name="ps_w", bufs=2, space="PSUM") as psw,
        ):
            # ---------------- persistent tiles + input DMAs ----------------
            wqTs = pp.tile([128, NDC, D], BF)
            wkTs = pp.tile([128, NDC, D], BF)
            xTs = pp.tile([128, NDC, T], BF)
            bias = pp.tile([128, D], BF)
            idts = pp.tile([128, 128], BF)
            catT = [pp.tile([128, TQ], BF, name=f"catT{g}") for g in range(8)]

            def wdma(dst, src, cb):
                nc.gpsimd.dma_start(
                    out=dst[:, 0:NDC, cb * 128:(cb + 1) * 128], in_=src[cb])

            def xdma(tb):
                # 4 chunked DMAs per block run on parallel DMA engines
                for ch in range(4):
                    dcs = slice(ch * 2, (ch + 1) * 2)
                    nc.gpsimd.dma_start(
                        out=xTs[:, dcs, tb * 512:(tb + 1) * 512],
                        in_=xt[tb][:, dcs, :])

            # proj g0 + xt blk0 first so PE can start early
            wdma(wqTs, wqt, 0)
            xdma(0)
            wdma(wkTs, wkt, 0)
            xdma(1)
            xdma(2)
            xdma(3)
            wvTs = wsh.tile([128, NDC, D], BF, tag="wvo", name="wv")
            wdma(wvTs, wvt, 0)
            nc.gpsimd.dma_start(out=idts[:, :], in_=idt[0:128])
            nc.gpsimd.dma_start(
                out=bias, in_=bass.AP(tensor=bo, offset=0, ap=[[0, 128], [1, D]]))
            for cb in range(1, 8):
                wdma(wqTs, wqt, cb)
                wdma(wkTs, wkt, cb)
                wdma(wvTs, wvt, cb)

            # ---------------- filler units (alloc at emission time) --------
            fq = []              # FIFO of (key, cost, unit_fn)
            pending = {}         # key -> units not yet emitted
            paid = [0]           # PE cycles of emitted filler work

            def queue_units(key, fns, cost):
                pending[key] = pending.get(key, 0) + len(fns)
                fq.extend((key, cost, f) for f in fns)

            def pop_one():
                key, cost, f = fq.pop(0)
                f()
                pending[key] -= 1
                paid[0] += cost

            def pop_to(target):
                while paid[0] < target and fq:
                    pop_one()

            def drain(key, leave=0):
                while pending.get(key, 0) > leave:
                    pop_one()

            # ---------------- V projection per head-pair ----------------
            vt_tiles = {}

            def v_units(pr):
                cs = slice(pr * 128, (pr + 1) * 128)
                split = pr >= 2
                nsc = 8 if split else NSC

                def unit(sc, pr=pr, cs=cs, split=split, nsc=nsc):
                    if sc == 0:
                        vt_tiles[pr] = vtp.tile([128, NSC, 2, 65], BF, tag="vt",
                                                name=f"vt{pr}")
                        nc.vector.memset(vt_tiles[pr][:, :, :, 64:65], 1.0)
                    p = psw.tile([128, 512], F32, tag="work")
                    for dc in range(NDC):
                        nc.tensor.matmul(
                            p[:, 0:128], xTs[:, dc, sc * 128:(sc + 1) * 128],
                            wvTs[:, dc, cs],
                            start=(dc == 0), stop=(dc == NDC - 1))
                    nc.vector.tensor_copy(
                        out=vt_tiles[pr][:, sc, :, 0:64],
                        in_=p[:, 0:128].rearrange("p (h c) -> p h c", h=2))
                    if split and sc == nsc - 1:
                        vt = vt_tiles[pr]
                        nc.sync.dma_start(out=vso[pr], in_=vt[:, 0:8, :, :])
                        nc.gpsimd.collective_compute(
                            "AllGather", BYP, RG,
                            ins=[vso[pr]], outs=[vsi[2 * pr:2 * pr + 2]])
                        nc.sync.dma_start(out=vt[:, 0:8, :, :],
                                          in_=vsi[2 * pr])
                        nc.sync.dma_start(out=vt[:, 8:16, :, :],
                                          in_=vsi[2 * pr + 1])
                return [lambda sc=sc: unit(sc) for sc in range(nsc)]

            # ---------------- Q/K projection per 2-head group ----------------
            proj_tiles = {}
            BLOCKS = ((0, 0, "q"), (0, 1, "q"), (1, 0, "k"),
                      (1, 1, "k"), (1, 2, "k"), (1, 3, "k"))

            def proj_unit(g, i, nu):
                dsti, blk, w = BLOCKS[i]
                ws = wqTs if w == "q" else wkTs
                if i == 0:
                    proj_tiles[g] = (
                        qk.tile([128, TQ], BF, tag="qT", name=f"qT{g}"),
                        qk.tile([128, T], BF, tag="kT", name=f"kT{g}"))
                p = psw.tile([128, 512], F32, tag="work")
                for dc in range(NDC):
                    nc.tensor.matmul(
                        p, ws[:, dc, g * 128:(g + 1) * 128],
                        xTs[:, dc, blk * 512:(blk + 1) * 512],
                        start=(dc == 0), stop=(dc == NDC - 1))
                nc.vector.tensor_copy(
                    out=proj_tiles[g][dsti][:, blk * 512:(blk + 1) * 512], in_=p)
                if nu == 4 and i == 3:
                    # split group: gather the other K half from the pair
                    kT = proj_tiles[g][1]
                    nc.sync.dma_start(out=kso[g], in_=kT[:, 0:TQ])
                    nc.gpsimd.collective_compute(
                        "AllGather", BYP, RG,
                        ins=[kso[g]], outs=[ksi[2 * g:2 * g + 2]])
                    nc.sync.dma_start(out=kT[:, 0:TQ], in_=ksi[2 * g])
                    nc.sync.dma_start(out=kT[:, TQ:T], in_=ksi[2 * g + 1])

            def proj_units(g, lo=0):
                nu = 4 if g >= 2 else 6
                return [lambda i=i, nu=nu: proj_unit(g, i, nu)
                        for i in range(lo, nu)]

            # ---------------- out-projection (tail) ----------------
            def outproj(qb, woTs):
                for nh in range(2):
                    p = psw.tile([128, 512], F32, tag="work", name=f"op{qb}{nh}")
                    for g in range(8):
                        nc.tensor.matmul(
                            p, catT[g][:, qb * 128:(qb + 1) * 128],
                            woTs[:, g, nh * 512:(nh + 1) * 512],
                            start=(g == 0), stop=(g == 7))
                    yt = yp.tile([128, 512], F32, tag="yt", name=f"yt{qb}{nh}")
                    nc.vector.tensor_add(
                        out=yt[:, :], in0=p, in1=bias[:, nh * 512:(nh + 1) * 512])
                    nc.sync.dma_start(
                        out=y[qb * 128:(qb + 1) * 128, nh * 512:(nh + 1) * 512],
                        in_=yt[:, :])

            # ---------------- PV sweep (reoriented) ----------------
            pts = {}        # (h, sc) -> pt tile
            catq_cur = {}   # g -> catq tile

            def pv_sweep(hp, qc):
                g, b = hp // 2, hp % 2
                if b == 0 and qc == 0:
                    catq_cur[g] = cq.tile([128, NQC, 2, 64], BF, tag="catq",
                                          name=f"catq{g}")
                pv = pspv.tile([128, 512], F32, tag="pv")
                vtq = vt_tiles[hp // 2]
                for sc in range(NSC):
                    nc.tensor.matmul(
                        pv[:, 0:65], pts[(hp, sc)][:, qc * 128:(qc + 1) * 128],
                        vtq[:, sc, b, :], start=(sc == 0), stop=(sc == NSC - 1))
                rec = rcp.tile([128, 1], F32, tag="rec")
                nc.vector.reciprocal_approx_fast(out=rec[:, :], in_=pv[:, 64:65])
                nc.vector.tensor_scalar(
                    out=catq_cur[g][:, qc, b, :], in0=pv[:, 0:64],
                    scalar1=rec[:, 0:1], scalar2=None, op0=MUL)

            def transpose_group(g):
                tp = psw.tile([128, 512], F32, tag="work", name=f"tp{g}")
                tpb = tp[:, :].bitcast(BF)   # [128, 1024] bf16 view
                for qc in range(NQC):
                    nc.tensor.transpose(
                        tpb[:, qc * 128:(qc + 1) * 128],
                        catq_cur[g][:, qc, :, :], idts)
                nc.vector.tensor_copy(out=catT[g][:, :], in_=tpb)

            # ---------------- prologue: qT0, qT1, kT0 of group 0 ------------
            for i in range(3):
                proj_unit(0, i, 6)

            # queue order matches deadline order; all WAR-safe at queue time
            queue_units("p0kt", proj_units(0, lo=3), P_COST)   # kT1..3
            queue_units("vp0", v_units(0), V_COST)
            queue_units("proj1", proj_units(1), P_COST)
            queue_units("vp1", v_units(1), V_COST)
            queue_units("proj2", proj_units(2), P_COST)
            queue_units("vp2", v_units(2), V_COST)

            # ---------------- attention main loop ----------------
            gsc = [0]   # cumulative pacing target
            woTs_h = [None]

            for h in range(NH):
                g, b = h // 2, h % 2
                # queue filler work at WAR-safe emission points
                if b == 1 and h >= 3 and (h - 3) // 2 + 3 <= 7:
                    pr = (h - 3) // 2 + 3
                    queue_units(f"vp{pr}", v_units(pr), V_COST)
                if b == 0 and h >= 2 and g + 2 <= 7:
                    queue_units(f"proj{g + 2}", proj_units(g + 2), P_COST)
                # deadline drains; split groups/pairs drain a step early so
                # the pairwise gather round-trip lands before first use
                if h >= 2:
                    drain(f"proj{g}")
                if b == 1 and 2 <= (h + 1) // 2 <= 7:
                    drain(f"proj{(h + 1) // 2}")
                if b == 0 and h >= 4:
                    drain(f"vp{h // 2}")
                if h == NH - 1:
                    # emit all remaining V work, then load Wo into Wv's slot
                    # early enough to cover the out-proj tail
                    while fq:
                        pop_one()
                    woTs_h[0] = wsh.tile([128, NDC, D], BF, tag="wvo", name="wo")
                    for dc in range(NDC):
                        nc.gpsimd.dma_start(out=woTs_h[0][:, dc, :], in_=wot[dc])
                qTg, kTg = proj_tiles[g]
                for sc in range(NSC):
                    if h == 0 and sc in (3, 7, 11):
                        drain("p0kt", leave=2 - (sc - 3) // 4)
                    if b == 1 and sc == 8:
                        drain(f"vp{g}")
                    lg = pslg.tile([128, 2, 512], F32, tag="lg")
                    for i in range(2):
                        nc.tensor.matmul(
                            lg[:, i, :],
                            kTg[64 * b:64 * b + 64, sc * 128:(sc + 1) * 128],
                            qTg[64 * b:64 * b + 64, i * 512:(i + 1) * 512],
                            start=True, stop=True)
                    pt = ptp.tile([128, TQ], BF, tag="pt", name=f"pt{h}_{sc}")
                    pts[(h, sc)] = pt
                    if sc in dve_sc(h):
                        nc.vector.tensor_scalar(
                            out=pt[:, :].bitcast(U16),
                            in0=lg.rearrange("p a b -> p (a b)"),
                            scalar1=SCH_A, scalar2=SCH_B, op0=MUL, op1=ADD)
                    else:
                        nc.scalar.activation(
                            out=pt[:, :], in_=lg.rearrange("p a b -> p (a b)"),
                            func=EXP, scale=SCALE)
                    if h > 0 and sc >= 8:
                        pv_sweep(h - 1, sc - 8)
                        if sc == NSC - 1 and (h - 1) % 2 == 1:
                            transpose_group((h - 1) // 2)
                    gsc[0] += PACE_A if sc < 8 else PACE_B
                    pop_to(gsc[0])

            # ---------------- tail: PV(15) / transpose g7 / out-proj --------
            # per-qc software pipeline (depth 2) so the PE never waits on the
            # DVE/Pool normalize+copy handoffs; transpose PSUM reuses the
            # now-dead lg pool bank pair
            tp_lg = pslg.tile([128, 2, 512], F32, tag="lg")
            tpb = tp_lg.rearrange("p a b -> p (a b)").bitcast(BF)

            def tail_tp(qc):
                nc.tensor.transpose(
                    tpb[:, qc * 128:(qc + 1) * 128],
                    catq_cur[7][:, qc, :, :], idts)
                nc.vector.tensor_copy(
                    out=catT[7][:, qc * 128:(qc + 1) * 128],
                    in_=tpb[:, qc * 128:(qc + 1) * 128])

            for qc in range(NQC):
                pv_sweep(NH - 1, qc)
                if qc >= 1:
                    tail_tp(qc - 1)
                if qc >= 2:
                    outproj(qc - 2, woTs_h[0])
            tail_tp(7)
            outproj(6, woTs_h[0])
            outproj(7, woTs_h[0])

    nc.compile()
    return nc


def make_in_maps(x, wq2, wk2, wv2, wo2, bo2):
    """Per-core input dicts from full (already 2-D) fp32 arrays."""
    bf = ml_dtypes.bfloat16

    def wblk(w2):
        # [colblk 8, 128 dkpart, NDC, 128 col]
        return np.ascontiguousarray(
            w2.T.astype(bf).reshape(NDC, 128, 8, 128).transpose(2, 1, 0, 3))

    wqt = wblk(wq2)
    wkt = wblk(wk2)
    wvt = wblk(wv2)
    wot = np.ascontiguousarray(wo2.T.astype(bf).reshape(NDC, 128, D))
    bo3 = np.ascontiguousarray(bo2.reshape(1, D).astype(bf))
    idt = np.eye(128, dtype=bf)
    in_maps = []
    for c in range(8):
        b, h = c // 2, c % 2
        xr = x[b] if h == 0 else np.concatenate([x[b, TQ:], x[b, :TQ]], axis=0)
        # [tblk 4, 128 dkpart, NDC, 512 t]
        xtc = np.ascontiguousarray(
            xr.T.astype(bf).reshape(NDC, 128, 4, 512).transpose(2, 1, 0, 3))
        in_maps.append({"xt": xtc, "wqt": wqt, "wkt": wkt, "wvt": wvt,
                        "wot": wot, "bo": bo3, "idt": idt})
    return in_maps


_CACHE = {}


def kernel(x, Wq, Wk, Wv, Wo, bo):
    if "nc" not in _CACHE:
        _CACHE["nc"] = build_nc()
    nc = _CACHE["nc"]
    x = np.ascontiguousarray(x, dtype=np.float32)
    in_maps = make_in_maps(
        x, np.asarray(Wq).reshape(D, D).astype(np.float32),
        np.asarray(Wk).reshape(D, D).astype(np.float32),
        np.asarray(Wv).reshape(D, D).astype(np.float32),
        np.asarray(Wo).astype(np.float32),
        np.asarray(bo).astype(np.float32))
    res = run_bass_kernel_spmd(nc, in_maps, core_ids=list(range(8)))
    out = np.empty((4, T, D), dtype=np.float32)
    for c in range(8):
        b, h = c // 2, c % 2
        out[b, h * TQ:(h + 1) * TQ] = res.results[c]["y"]
    return out


# revision 32
# speedup vs baseline: 1.3532x; 1.3532x over previous
"""MHA kernel for TRN2: B=4,T=2048,D=1024,H=16,HD=64 across 8 NeuronCores.

Sharding: core c -> batch c//2, query half c%2 (host rotates the sequence so
each core's queries are rows 0:1024; softmax over keys is permutation
invariant). No collectives.

Design (v3 -- reoriented PV + cycle-paced fillers):
- bf16 everywhere; x^T and W^T are pre-transposed/cast/regrouped on the
  host so every weight slice arrives in one large DMA.
- Logits per (head, s-chunk): out [128 s, 2x512 q] from K-stationary
  matmuls (contraction hd=64 via base-partition 64/0 tiles).
- exp on ScalarE over [128, 1024] tiles -> pt ring (32 slots) in SBUF.
- PV is REORIENTED: probs are the stationary operand [128 s, 128 q],
  V (+ones col) the moving [128 s, 65] -> out [128 q, 65] accumulated over
  all of s into a single PSUM bank per q-chunk (qc-outer sweep, one open
  accumulation group at a time, swept during the next head's step).
  This runs PV at full PE efficiency: 262k -> 133k PE cycles.
- Per-q denominators land in pv[:, 64] (ones column); normalization is a
  DVE reciprocal_approx + per-partition-scalar multiply on GpSimd (q is on
  partitions now).
- cat comes out [q, hd]; per group of 2 heads it is transposed back to
  [hd2, q] with 8 PE-transpose ops into one PSUM bank, then one copy.
- V is computed per head-pair just-in-time (vt pool of 3 pair tiles keeps
  SBUF small enough for the 32-slot pt ring); Wo reuses Wv's SBUF slot.
- Projection/V work is queued as ~1k/4k-cycle filler units at WAR-safe
  points and popped under a cycle-paced budget (~1.3k PE cycles per sc
  step) so TensorE stays saturated through the whole ACT-bound midgame;
  per-unit PSUM work tiles are allocated at emission time so pool rotation
  order matches the instruction stream.
"""
import sys
sys.path.insert(0, "/opt/trn_rl_repo")
import warnings
warnings.filterwarnings("ignore")

import numpy as np
import ml_dtypes
import concourse.bass as bass
import concourse.mybir as mybir
import concourse.tile as tile
from concourse import bacc
from concourse.bass_utils import run_bass_kernel_spmd

F32 = mybir.dt.float32
BF = mybir.dt.bfloat16
EXP = mybir.ActivationFunctionType.Exp
MUL = mybir.AluOpType.mult

T, D = 2048, 1024
TQ = 1024          # queries per core
NH = 16            # heads
NSC = 16           # s chunks of 128
NDC = 8            # d chunks of 128
NQC = 8            # q chunks of 128
SCALE = 0.125      # 1/sqrt(64)

V_COST, P_COST = 1024, 4096    # PE cycles per filler unit
PACE_A, PACE_B = 1500, 1050    # filler pace (cycles/sc): first/second half

# Schraudolph exp on DVE for a few s-chunks per head: exp(s*x) ~=
# bitcast_f32(int32(x*A + B)); emitting uint16(x*A/2^16 + B/2^16) yields
# the TOP half of that fp32 word, i.e. the bf16 bit pattern, directly into
# the regular bf16 pt ring. ~1.5% rms prob error on those chunks, which
# the softmax tolerates (measured end-to-end ~4e-3 rel).
SCH_A = float(1 << 23) * 1.4426950408889634 * SCALE / 65536.0
SCH_B = 128.0 * (127.0 - 0.0440)
U16 = mybir.dt.uint16
ADD = mybir.AluOpType.add


def dve_sc(h):
    # step 15 runs filler-dry, so shift more exp load off ACT there
    return (3, 5, 7, 9, 11, 13, 15) if h == NH - 1 else (5, 13)


def build_nc():
    nc = bacc.Bacc("TRN2", target_bir_lowering=False, debug=False, num_devices=8)
    # xt: [4 tblk, 128, NDC, 512]; wq/wk/wv: [8 colblk, 128, NDC, 128]
    xt = nc.dram_tensor("xt", [4, 128, NDC, 512], BF, kind="ExternalInput")
    wqt = nc.dram_tensor("wqt", [8, 128, NDC, 128], BF, kind="ExternalInput")
    wkt = nc.dram_tensor("wkt", [8, 128, NDC, 128], BF, kind="ExternalInput")
    wvt = nc.dram_tensor("wvt", [8, 128, NDC, 128], BF, kind="ExternalInput")
    wot = nc.dram_tensor("wot", [NDC, 128, D], BF, kind="ExternalInput")
    bo = nc.dram_tensor("bo", [1, D], BF, kind="ExternalInput")
    idt = nc.dram_tensor("idt", [128, 128], BF, kind="ExternalInput")
    y = nc.dram_tensor("y", [TQ, D], F32, kind="ExternalOutput")

    with tile.TileContext(nc) as tc:
        with (
            tc.tile_pool(name="persist", bufs=1) as pp,
            tc.tile_pool(name="wsh", bufs=1) as wsh,
            tc.tile_pool(name="vtp", bufs=3) as vtp,
            tc.tile_pool(name="qk", bufs=3) as qk,
            tc.tile_pool(name="ptp", bufs=32) as ptp,
            tc.tile_pool(name="cq", bufs=2) as cq,
            tc.tile_pool(name="rcp", bufs=8) as rcp,
            tc.tile_pool(name="yp", bufs=2) as yp,
            tc.tile_pool(name="ps_lg", bufs=2, space="PSUM") as pslg,
            tc.tile_pool(name="ps_pv", bufs=2, space="PSUM") as pspv,
            tc.tile_pool(name="ps_w", bufs=2, space="PSUM") as psw,
        ):
            # ---------------- persistent tiles + input DMAs ----------------
            wqTs = pp.tile([128, NDC, D], BF)
            wkTs = pp.tile([128, NDC, D], BF)
            xTs = pp.tile([128, NDC, T], BF)
            bias = pp.tile([128, D], BF)
            idts = pp.tile([128, 128], BF)
            catT = [pp.tile([128, TQ], BF, name=f"catT{g}") for g in range(8)]

            def wdma(dst, src, cb):
                nc.gpsimd.dma_start(
                    out=dst[:, 0:NDC, cb * 128:(cb + 1) * 128], in_=src[cb])

            def xdma(tb):
                # 4 chunked DMAs per block run on parallel DMA engines
                for ch in range(4):
                    dcs = slice(ch * 2, (ch + 1) * 2)
                    nc.gpsimd.dma_start(
                        out=xTs[:, dcs, tb * 512:(tb + 1) * 512],
                        in_=xt[tb][:, dcs, :])

            # proj g0 + xt blk0 first so PE can start early
            wdma(wqTs, wqt, 0)
            xdma(0)
            wdma(wkTs, wkt, 0)
            xdma(1)
            xdma(2)
            xdma(3)
            wvTs = wsh.tile([128, NDC, D], BF, tag="wvo", name="wv")
            wdma(wvTs, wvt, 0)
            nc.gpsimd.dma_start(out=idts[:, :], in_=idt[0:128])
            nc.gpsimd.dma_start(
                out=bias, in_=bass.AP(tensor=bo, offset=0, ap=[[0, 128], [1, D]]))
            for cb in range(1, 8):
                wdma(wqTs, wqt, cb)
                wdma(wkTs, wkt, cb)
                wdma(wvTs, wvt, cb)

            # ---------------- filler units (alloc at emission time) --------
            fq = []              # FIFO of (key, cost, unit_fn)
            pending = {}         # key -> units not yet emitted
            paid = [0]           # PE cycles of emitted filler work

            def queue_units(key, fns, cost):
                pending[key] = pending.get(key, 0) + len(fns)
                fq.extend((key, cost, f) for f in fns)

            def pop_one():
                key, cost, f = fq.pop(0)
                f()
                pending[key] -= 1
                paid[0] += cost

            def pop_to(target):
                while paid[0] < target and fq:
                    pop_one()

            def drain(key, leave=0):
                while pending.get(key, 0) > leave:
                    pop_one()

            # ---------------- V projection per head-pair ----------------
            vt_tiles = {}

            def v_units(pr):
                cs = slice(pr * 128, (pr + 1) * 128)

                def unit(sc, pr=pr, cs=cs):
                    if sc == 0:
                        vt_tiles[pr] = vtp.tile([128, NSC, 2, 65], BF, tag="vt",
                                                name=f"vt{pr}")
                        nc.vector.memset(vt_tiles[pr][:, :, :, 64:65], 1.0)
                    p = psw.tile([128, 512], F32, tag="work")
                    for dc in range(NDC):
                        nc.tensor.matmul(
                            p[:, 0:128], xTs[:, dc, sc * 128:(sc + 1) * 128],
                            wvTs[:, dc, cs],
                            start=(dc == 0), stop=(dc == NDC - 1))
                    nc.vector.tensor_copy(
                        out=vt_tiles[pr][:, sc, :, 0:64],
                        in_=p[:, 0:128].rearrange("p (h c) -> p h c", h=2))
                return [lambda sc=sc: unit(sc) for sc in range(NSC)]

            # ---------------- Q/K projection per 2-head group ----------------
            proj_tiles = {}
            BLOCKS = ((0, 0, "q"), (0, 1, "q"), (1, 0, "k"),
                      (1, 1, "k"), (1, 2, "k"), (1, 3, "k"))

            def proj_unit(g, i):
                dsti, blk, w = BLOCKS[i]
                ws = wqTs if w == "q" else wkTs
                if i == 0:
                    proj_tiles[g] = (
                        qk.tile([128, TQ], BF, tag="qT", name=f"qT{g}"),
                        qk.tile([128, T], BF, tag="kT", name=f"kT{g}"))
                p = psw.tile([128, 512], F32, tag="work")
                for dc in range(NDC):
                    nc.tensor.matmul(
                        p, ws[:, dc, g * 128:(g + 1) * 128],
                        xTs[:, dc, blk * 512:(blk + 1) * 512],
                        start=(dc == 0), stop=(dc == NDC - 1))
                nc.vector.tensor_copy(
                    out=proj_tiles[g][dsti][:, blk * 512:(blk + 1) * 512], in_=p)

            def proj_units(g, lo=0):
                return [lambda i=i: proj_unit(g, i) for i in range(lo, 6)]

            # ---------------- out-projection (tail) ----------------
            def outproj(qb, woTs):
                for nh in range(2):
                    p = psw.tile([128, 512], F32, tag="work", name=f"op{qb}{nh}")
                    for g in range(8):
                        nc.tensor.matmul(
                            p, catT[g][:, qb * 128:(qb + 1) * 128],
                            woTs[:, g, nh * 512:(nh + 1) * 512],
                            start=(g == 0), stop=(g == 7))
                    yt = yp.tile([128, 512], F32, tag="yt", name=f"yt{qb}{nh}")
                    nc.vector.tensor_add(
                        out=yt[:, :], in0=p, in1=bias[:, nh * 512:(nh + 1) * 512])
                    nc.sync.dma_start(
                        out=y[qb * 128:(qb + 1) * 128, nh * 512:(nh + 1) * 512],
                        in_=yt[:, :])

            # ---------------- PV sweep (reoriented) ----------------
            pts = {}        # (h, sc) -> pt tile
            catq_cur = {}   # g -> catq tile

            def pv_sweep(hp, qc):
                g, b = hp // 2, hp % 2
                if b == 0 and qc == 0:
                    catq_cur[g] = cq.tile([128, NQC, 2, 64], BF, tag="catq",
                                          name=f"catq{g}")
                pv = pspv.tile([128, 512], F32, tag="pv")
                vtq = vt_tiles[hp // 2]
                for sc in range(NSC):
                    nc.tensor.matmul(
                        pv[:, 0:65], pts[(hp, sc)][:, qc * 128:(qc + 1) * 128],
                        vtq[:, sc, b, :], start=(sc == 0), stop=(sc == NSC - 1))
                rec = rcp.tile([128, 1], F32, tag="rec")
                nc.vector.reciprocal_approx_fast(out=rec[:, :], in_=pv[:, 64:65])
                nc.vector.tensor_scalar(
                    out=catq_cur[g][:, qc, b, :], in0=pv[:, 0:64],
                    scalar1=rec[:, 0:1], scalar2=None, op0=MUL)

            def transpose_group(g):
                tp = psw.tile([128, 512], F32, tag="work", name=f"tp{g}")
                tpb = tp[:, :].bitcast(BF)   # [128, 1024] bf16 view
                for qc in range(NQC):
                    nc.tensor.transpose(
                        tpb[:, qc * 128:(qc + 1) * 128],
                        catq_cur[g][:, qc, :, :], idts)
                nc.vector.tensor_copy(out=catT[g][:, :], in_=tpb)

            # ---------------- prologue: qT0, qT1, kT0 of group 0 ------------
            for i in range(3):
                proj_unit(0, i)

            # queue order matches deadline order; all WAR-safe at queue time
            queue_units("p0kt", proj_units(0, lo=3), P_COST)   # kT1..3
            queue_units("vp0", v_units(0), V_COST)
            queue_units("proj1", proj_units(1), P_COST)
            queue_units("vp1", v_units(1), V_COST)
            queue_units("proj2", proj_units(2), P_COST)
            queue_units("vp2", v_units(2), V_COST)

            # ---------------- attention main loop ----------------
            gsc = [0]   # cumulative pacing target
            woTs_h = [None]

            for h in range(NH):
                g, b = h // 2, h % 2
                # queue filler work at WAR-safe emission points
                if b == 1 and h >= 3 and (h - 3) // 2 + 3 <= 7:
                    pr = (h - 3) // 2 + 3
                    queue_units(f"vp{pr}", v_units(pr), V_COST)
                if b == 0 and h >= 2 and g + 2 <= 7:
                    queue_units(f"proj{g + 2}", proj_units(g + 2), P_COST)
                # deadline drains
                if h >= 2:
                    drain(f"proj{g}")
                if h == NH - 1:
                    # emit all remaining V work, then load Wo into Wv's slot
                    # early enough to cover the out-proj tail
                    while fq:
                        pop_one()
                    woTs_h[0] = wsh.tile([128, NDC, D], BF, tag="wvo", name="wo")
                    for dc in range(NDC):
                        nc.gpsimd.dma_start(out=woTs_h[0][:, dc, :], in_=wot[dc])
                qTg, kTg = proj_tiles[g]
                for sc in range(NSC):
                    if h == 0 and sc in (3, 7, 11):
                        drain("p0kt", leave=2 - (sc - 3) // 4)
                    if b == 1 and sc == 8:
                        drain(f"vp{g}")
                    lg = pslg.tile([128, 2, 512], F32, tag="lg")
                    for i in range(2):
                        nc.tensor.matmul(
                            lg[:, i, :],
                            kTg[64 * b:64 * b + 64, sc * 128:(sc + 1) * 128],
                            qTg[64 * b:64 * b + 64, i * 512:(i + 1) * 512],
                            start=True, stop=True)
                    pt = ptp.tile([128, TQ], BF, tag="pt", name=f"pt{h}_{sc}")
                    pts[(h, sc)] = pt
                    if sc in dve_sc(h):
                        nc.vector.tensor_scalar(
                            out=pt[:, :].bitcast(U16),
                            in0=lg.rearrange("p a b -> p (a b)"),
                            scalar1=SCH_A, scalar2=SCH_B, op0=MUL, op1=ADD)
                    else:
                        nc.scalar.activation(
                            out=pt[:, :], in_=lg.rearrange("p a b -> p (a b)"),
                            func=EXP, scale=SCALE)
                    if h > 0 and sc >= 8:
                        pv_sweep(h - 1, sc - 8)
                        if sc == NSC - 1 and (h - 1) % 2 == 1:
                            transpose_group((h - 1) // 2)
                    gsc[0] += PACE_A if sc < 8 else PACE_B
                    pop_to(gsc[0])

            # ---------------- tail: PV(15) / transpose g7 / out-proj --------
            # per-qc software pipeline (depth 2) so the PE never waits on the
            # DVE/Pool normalize+copy handoffs; transpose PSUM reuses the
            # now-dead lg pool bank pair
            tp_lg = pslg.tile([128, 2, 512], F32, tag="lg")
            tpb = tp_lg.rearrange("p a b -> p (a b)").bitcast(BF)

            def tail_tp(qc):
                nc.tensor.transpose(
                    tpb[:, qc * 128:(qc + 1) * 128],
                    catq_cur[7][:, qc, :, :], idts)
                nc.vector.tensor_copy(
                    out=catT[7][:, qc * 128:(qc + 1) * 128],
                    in_=tpb[:, qc * 128:(qc + 1) * 128])

            for qc in range(NQC):
                pv_sweep(NH - 1, qc)
                if qc >= 1:
                    tail_tp(qc - 1)
                if qc >= 2:
                    outproj(qc - 2, woTs_h[0])
            tail_tp(7)
            outproj(6, woTs_h[0])
            outproj(7, woTs_h[0])

    nc.compile()
    return nc


def make_in_maps(x, wq2, wk2, wv2, wo2, bo2):
    """Per-core input dicts from full (already 2-D) fp32 arrays."""
    bf = ml_dtypes.bfloat16

    def wblk(w2):
        # [colblk 8, 128 dkpart, NDC, 128 col]
        return np.ascontiguousarray(
            w2.T.astype(bf).reshape(NDC, 128, 8, 128).transpose(2, 1, 0, 3))

    wqt = wblk(wq2)
    wkt = wblk(wk2)
    wvt = wblk(wv2)
    wot = np.ascontiguousarray(wo2.T.astype(bf).reshape(NDC, 128, D))
    bo3 = np.ascontiguousarray(bo2.reshape(1, D).astype(bf))
    idt = np.eye(128, dtype=bf)
    in_maps = []
    for c in range(8):
        b, h = c // 2, c % 2
        xr = x[b] if h == 0 else np.concatenate([x[b, TQ:], x[b, :TQ]], axis=0)
        # [tblk 4, 128 dkpart, NDC, 512 t]
        xtc = np.ascontiguousarray(
            xr.T.astype(bf).reshape(NDC, 128, 4, 512).transpose(2, 1, 0, 3))
        in_maps.append({"xt": xtc, "wqt": wqt, "wkt": wkt, "wvt": wvt,
                        "wot": wot, "bo": bo3, "idt": idt})
    return in_maps


_CACHE = {}


def kernel(x, Wq, Wk, Wv, Wo, bo):
    if "nc" not in _CACHE:
        _CACHE["nc"] = build_nc()
    nc = _CACHE["nc"]
    x = np.ascontiguousarray(x, dtype=np.float32)
    in_maps = make_in_maps(
        x, np.asarray(Wq).reshape(D, D).astype(np.float32),
        np.asarray(Wk).reshape(D, D).astype(np.float32),
        np.asarray(Wv).reshape(D, D).astype(np.float32),
        np.asarray(Wo).astype(np.float32),
        np.asarray(bo).astype(np.float32))
    res = run_bass_kernel_spmd(nc, in_maps, core_ids=list(range(8)))
    out = np.empty((4, T, D), dtype=np.float32)
    for c in range(8):
        b, h = c // 2, c % 2
        out[b, h * TQ:(h + 1) * TQ] = res.results[c]["y"]
    return out


# revision 34
# speedup vs baseline: 1.3539x; 1.0006x over previous
"""MHA kernel for TRN2: B=4,T=2048,D=1024,H=16,HD=64 across 8 NeuronCores.

Sharding: core c -> batch c//2, query half c%2 (host rotates the sequence so
each core's queries are rows 0:1024; softmax over keys is permutation
invariant). No collectives.

Design (v3 -- reoriented PV + cycle-paced fillers):
- bf16 everywhere; x^T and W^T are pre-transposed/cast/regrouped on the
  host so every weight slice arrives in one large DMA.
- Logits per (head, s-chunk): out [128 s, 2x512 q] from K-stationary
  matmuls (contraction hd=64 via base-partition 64/0 tiles).
- exp on ScalarE over [128, 1024] tiles -> pt ring (32 slots) in SBUF.
- PV is REORIENTED: probs are the stationary operand [128 s, 128 q],
  V (+ones col) the moving [128 s, 65] -> out [128 q, 65] accumulated over
  all of s into a single PSUM bank per q-chunk (qc-outer sweep, one open
  accumulation group at a time, swept during the next head's step).
  This runs PV at full PE efficiency: 262k -> 133k PE cycles.
- Per-q denominators land in pv[:, 64] (ones column); normalization is a
  DVE reciprocal_approx + per-partition-scalar multiply on GpSimd (q is on
  partitions now).
- cat comes out [q, hd]; per group of 2 heads it is transposed back to
  [hd2, q] with 8 PE-transpose ops into one PSUM bank, then one copy.
- V is computed per head-pair just-in-time (vt pool of 3 pair tiles keeps
  SBUF small enough for the 32-slot pt ring); Wo reuses Wv's SBUF slot.
- Projection/V work is queued as ~1k/4k-cycle filler units at WAR-safe
  points and popped under a cycle-paced budget (~1.3k PE cycles per sc
  step) so TensorE stays saturated through the whole ACT-bound midgame;
  per-unit PSUM work tiles are allocated at emission time so pool rotation
  order matches the instruction stream.
"""
import sys
sys.path.insert(0, "/opt/trn_rl_repo")
import warnings
warnings.filterwarnings("ignore")

import numpy as np
import ml_dtypes
import concourse.bass as bass
import concourse.mybir as mybir
import concourse.tile as tile
from concourse import bacc
from concourse.bass_utils import run_bass_kernel_spmd

F32 = mybir.dt.float32
BF = mybir.dt.bfloat16
EXP = mybir.ActivationFunctionType.Exp
MUL = mybir.AluOpType.mult

T, D = 2048, 1024
TQ = 1024          # queries per core
NH = 16            # heads
NSC = 16           # s chunks of 128
NDC = 8            # d chunks of 128
NQC = 8            # q chunks of 128
SCALE = 0.125      # 1/sqrt(64)

V_COST, P_COST = 1024, 4096    # PE cycles per filler unit
PACE_A, PACE_B = 1500, 1050    # filler pace (cycles/sc): first/second half

# Schraudolph exp on DVE for a few s-chunks per head: exp(s*x) ~=
# bitcast_f32(int32(x*A + B)); emitting uint16(x*A/2^16 + B/2^16) yields
# the TOP half of that fp32 word, i.e. the bf16 bit pattern, directly into
# the regular bf16 pt ring. ~1.5% rms prob error on those chunks, which
# the softmax tolerates (measured end-to-end ~4e-3 rel).
SCH_A = float(1 << 23) * 1.4426950408889634 * SCALE / 65536.0
SCH_B = 128.0 * (127.0 - 0.0440)
U16 = mybir.dt.uint16
ADD = mybir.AluOpType.add


def dve_sc(h):
    # late steps run filler-dry, so shift more exp load off ACT there
    if h == NH - 1:
        return (3, 5, 7, 9, 11, 13, 15)
    if h >= NH - 3:
        return (1, 5, 9, 13)
    return (5, 13)


def build_nc():
    nc = bacc.Bacc("TRN2", target_bir_lowering=False, debug=False, num_devices=8)
    # xt: [4 tblk, 128, NDC, 512]; wq/wk/wv: [8 colblk, 128, NDC, 128]
    xt = nc.dram_tensor("xt", [4, 128, NDC, 512], BF, kind="ExternalInput")
    wqt = nc.dram_tensor("wqt", [8, 128, NDC, 128], BF, kind="ExternalInput")
    wkt = nc.dram_tensor("wkt", [8, 128, NDC, 128], BF, kind="ExternalInput")
    wvt = nc.dram_tensor("wvt", [8, 128, NDC, 128], BF, kind="ExternalInput")
    wot = nc.dram_tensor("wot", [NDC, 128, D], BF, kind="ExternalInput")
    bo = nc.dram_tensor("bo", [1, D], BF, kind="ExternalInput")
    idt = nc.dram_tensor("idt", [128, 128], BF, kind="ExternalInput")
    y = nc.dram_tensor("y", [TQ, D], F32, kind="ExternalOutput")

    with tile.TileContext(nc) as tc:
        with (
            tc.tile_pool(name="persist", bufs=1) as pp,
            tc.tile_pool(name="wsh", bufs=1) as wsh,
            tc.tile_pool(name="vtp", bufs=3) as vtp,
            tc.tile_pool(name="qk", bufs=3) as qk,
            tc.tile_pool(name="ptp", bufs=32) as ptp,
            tc.tile_pool(name="cq", bufs=2) as cq,
            tc.tile_pool(name="rcp", bufs=8) as rcp,
            tc.tile_pool(name="yp", bufs=2) as yp,
            tc.tile_pool(name="ps_lg", bufs=2, space="PSUM") as pslg,
            tc.tile_pool(name="ps_pv", bufs=2, space="PSUM") as pspv,
            tc.tile_pool(name="ps_w", bufs=2, space="PSUM") as psw,
        ):
            # ---------------- persistent tiles + input DMAs ----------------
            wqTs = pp.tile([128, NDC, D], BF)
            wkTs = pp.tile([128, NDC, D], BF)
            xTs = pp.tile([128, NDC, T], BF)
            bias = pp.tile([128, D], BF)
            idts = pp.tile([128, 128], BF)
            catT = [pp.tile([128, TQ], BF, name=f"catT{g}") for g in range(8)]

            def wdma(dst, src, cb):
                nc.gpsimd.dma_start(
                    out=dst[:, 0:NDC, cb * 128:(cb + 1) * 128], in_=src[cb])

            def xdma(tb):
                # 4 chunked DMAs per block run on parallel DMA engines
                for ch in range(4):
                    dcs = slice(ch * 2, (ch + 1) * 2)
                    nc.gpsimd.dma_start(
                        out=xTs[:, dcs, tb * 512:(tb + 1) * 512],
                        in_=xt[tb][:, dcs, :])

            # proj g0 + xt blk0 first, in small chunks on parallel DMA
            # engines, so PE can start early
            for ch in range(4):
                dcs = slice(ch * 2, (ch + 1) * 2)
                nc.gpsimd.dma_start(
                    out=wqTs[:, dcs, 0:128], in_=wqt[0][:, dcs, :])
            xdma(0)
            for ch in range(4):
                dcs = slice(ch * 2, (ch + 1) * 2)
                nc.gpsimd.dma_start(
                    out=wkTs[:, dcs, 0:128], in_=wkt[0][:, dcs, :])
            xdma(1)
            xdma(2)
            xdma(3)
            wvTs = wsh.tile([128, NDC, D], BF, tag="wvo", name="wv")
            wdma(wvTs, wvt, 0)
            nc.gpsimd.dma_start(out=idts[:, :], in_=idt[0:128])
            nc.gpsimd.dma_start(
                out=bias, in_=bass.AP(tensor=bo, offset=0, ap=[[0, 128], [1, D]]))
            for cb in range(1, 8):
                wdma(wqTs, wqt, cb)
                wdma(wkTs, wkt, cb)
                wdma(wvTs, wvt, cb)

            # ---------------- filler units (alloc at emission time) --------
            fq = []              # FIFO of (key, cost, unit_fn)
            pending = {}         # key -> units not yet emitted
            paid = [0]           # PE cycles of emitted filler work

            def queue_units(key, fns, cost):
                pending[key] = pending.get(key, 0) + len(fns)
                fq.extend((key, cost, f) for f in fns)

            def pop_one():
                key, cost, f = fq.pop(0)
                f()
                pending[key] -= 1
                paid[0] += cost

            def pop_to(target):
                while paid[0] < target and fq:
                    pop_one()

            def drain(key, leave=0):
                while pending.get(key, 0) > leave:
                    pop_one()

            # ---------------- V projection per head-pair ----------------
            vt_tiles = {}

            def v_units(pr):
                cs = slice(pr * 128, (pr + 1) * 128)

                def unit(sc, pr=pr, cs=cs):
                    if sc == 0:
                        vt_tiles[pr] = vtp.tile([128, NSC, 2, 65], BF, tag="vt",
                                                name=f"vt{pr}")
                        nc.vector.memset(vt_tiles[pr][:, :, :, 64:65], 1.0)
                    p = psw.tile([128, 512], F32, tag="work")
                    for dc in range(NDC):
                        nc.tensor.matmul(
                            p[:, 0:128], xTs[:, dc, sc * 128:(sc + 1) * 128],
                            wvTs[:, dc, cs],
                            start=(dc == 0), stop=(dc == NDC - 1))
                    nc.vector.tensor_copy(
                        out=vt_tiles[pr][:, sc, :, 0:64],
                        in_=p[:, 0:128].rearrange("p (h c) -> p h c", h=2))
                return [lambda sc=sc: unit(sc) for sc in range(NSC)]

            # ---------------- Q/K projection per 2-head group ----------------
            proj_tiles = {}
            BLOCKS = ((0, 0, "q"), (0, 1, "q"), (1, 0, "k"),
                      (1, 1, "k"), (1, 2, "k"), (1, 3, "k"))

            def proj_unit(g, i):
                dsti, blk, w = BLOCKS[i]
                ws = wqTs if w == "q" else wkTs
                if i == 0:
                    proj_tiles[g] = (
                        qk.tile([128, TQ], BF, tag="qT", name=f"qT{g}"),
                        qk.tile([128, T], BF, tag="kT", name=f"kT{g}"))
                p = psw.tile([128, 512], F32, tag="work")
                for dc in range(NDC):
                    nc.tensor.matmul(
                        p, ws[:, dc, g * 128:(g + 1) * 128],
                        xTs[:, dc, blk * 512:(blk + 1) * 512],
                        start=(dc == 0), stop=(dc == NDC - 1))
                nc.vector.tensor_copy(
                    out=proj_tiles[g][dsti][:, blk * 512:(blk + 1) * 512], in_=p)

            def proj_units(g, lo=0):
                return [lambda i=i: proj_unit(g, i) for i in range(lo, 6)]

            # ---------------- out-projection (tail) ----------------
            def outproj(qb, woTs):
                for nh in range(2):
                    p = psw.tile([128, 512], F32, tag="work", name=f"op{qb}{nh}")
                    for g in range(8):
                        nc.tensor.matmul(
                            p, catT[g][:, qb * 128:(qb + 1) * 128],
                            woTs[:, g, nh * 512:(nh + 1) * 512],
                            start=(g == 0), stop=(g == 7))
                    yt = yp.tile([128, 512], F32, tag="yt", name=f"yt{qb}{nh}")
                    nc.vector.tensor_add(
                        out=yt[:, :], in0=p, in1=bias[:, nh * 512:(nh + 1) * 512])
                    nc.sync.dma_start(
                        out=y[qb * 128:(qb + 1) * 128, nh * 512:(nh + 1) * 512],
                        in_=yt[:, :])

            # ---------------- PV sweep (reoriented) ----------------
            pts = {}        # (h, sc) -> pt tile
            catq_cur = {}   # g -> catq tile

            def pv_sweep(hp, qc):
                g, b = hp // 2, hp % 2
                if b == 0 and qc == 0:
                    catq_cur[g] = cq.tile([128, NQC, 2, 64], BF, tag="catq",
                                          name=f"catq{g}")
                pv = pspv.tile([128, 512], F32, tag="pv")
                vtq = vt_tiles[hp // 2]
                for sc in range(NSC):
                    nc.tensor.matmul(
                        pv[:, 0:65], pts[(hp, sc)][:, qc * 128:(qc + 1) * 128],
                        vtq[:, sc, b, :], start=(sc == 0), stop=(sc == NSC - 1))
                rec = rcp.tile([128, 1], F32, tag="rec")
                nc.vector.reciprocal_approx_fast(out=rec[:, :], in_=pv[:, 64:65])
                nc.vector.tensor_scalar(
                    out=catq_cur[g][:, qc, b, :], in0=pv[:, 0:64],
                    scalar1=rec[:, 0:1], scalar2=None, op0=MUL)

            def transpose_group(g):
                tp = psw.tile([128, 512], F32, tag="work", name=f"tp{g}")
                tpb = tp[:, :].bitcast(BF)   # [128, 1024] bf16 view
                for qc in range(NQC):
                    nc.tensor.transpose(
                        tpb[:, qc * 128:(qc + 1) * 128],
                        catq_cur[g][:, qc, :, :], idts)
                nc.vector.tensor_copy(out=catT[g][:, :], in_=tpb)

            # ---------------- prologue: qT0, qT1, kT0 of group 0 ------------
            for i in range(3):
                proj_unit(0, i)

            # queue order matches deadline order; all WAR-safe at queue time
            queue_units("p0kt", proj_units(0, lo=3), P_COST)   # kT1..3
            queue_units("vp0", v_units(0), V_COST)
            queue_units("proj1", proj_units(1), P_COST)
            queue_units("vp1", v_units(1), V_COST)
            queue_units("proj2", proj_units(2), P_COST)
            queue_units("vp2", v_units(2), V_COST)

            # ---------------- attention main loop ----------------
            gsc = [0]   # cumulative pacing target
            woTs_h = [None]

            for h in range(NH):
                g, b = h // 2, h % 2
                # queue filler work at WAR-safe emission points
                if b == 1 and h >= 3 and (h - 3) // 2 + 3 <= 7:
                    pr = (h - 3) // 2 + 3
                    queue_units(f"vp{pr}", v_units(pr), V_COST)
                if b == 0 and h >= 2 and g + 2 <= 7:
                    queue_units(f"proj{g + 2}", proj_units(g + 2), P_COST)
                # deadline drains
                if h >= 2:
                    drain(f"proj{g}")
                if h == NH - 1:
                    # emit all remaining V work, then load Wo into Wv's slot
                    # early enough to cover the out-proj tail
                    while fq:
                        pop_one()
                    woTs_h[0] = wsh.tile([128, NDC, D], BF, tag="wvo", name="wo")
                    for dc in range(NDC):
                        nc.gpsimd.dma_start(out=woTs_h[0][:, dc, :], in_=wot[dc])
                qTg, kTg = proj_tiles[g]
                for sc in range(NSC):
                    if h == 0 and sc in (3, 7, 11):
                        drain("p0kt", leave=2 - (sc - 3) // 4)
                    if b == 1 and sc == 8:
                        drain(f"vp{g}")
                    lg = pslg.tile([128, 2, 512], F32, tag="lg")
                    for i in range(2):
                        nc.tensor.matmul(
                            lg[:, i, :],
                            kTg[64 * b:64 * b + 64, sc * 128:(sc + 1) * 128],
                            qTg[64 * b:64 * b + 64, i * 512:(i + 1) * 512],
                            start=True, stop=True)
                    pt = ptp.tile([128, TQ], BF, tag="pt", name=f"pt{h}_{sc}")
                    pts[(h, sc)] = pt
                    if sc in dve_sc(h):
                        nc.vector.tensor_scalar(
                            out=pt[:, :].bitcast(U16),
                            in0=lg.rearrange("p a b -> p (a b)"),
                            scalar1=SCH_A, scalar2=SCH_B, op0=MUL, op1=ADD)
                    else:
                        nc.scalar.activation(
                            out=pt[:, :], in_=lg.rearrange("p a b -> p (a b)"),
                            func=EXP, scale=SCALE)
                    if h > 0 and sc >= 8:
                        pv_sweep(h - 1, sc - 8)
                        if sc == NSC - 1 and (h - 1) % 2 == 1:
                            transpose_group((h - 1) // 2)
                    gsc[0] += PACE_A if sc < 8 else PACE_B
                    pop_to(gsc[0])

            # ---------------- tail: PV(15) / transpose g7 / out-proj --------
            # per-qc software pipeline (depth 2) so the PE never waits on the
            # DVE/Pool normalize+copy handoffs; transpose PSUM reuses the
            # now-dead lg pool bank pair
            tp_lg = pslg.tile([128, 2, 512], F32, tag="lg")
            tpb = tp_lg.rearrange("p a b -> p (a b)").bitcast(BF)

            def tail_tp(qc):
                nc.tensor.transpose(
                    tpb[:, qc * 128:(qc + 1) * 128],
                    catq_cur[7][:, qc, :, :], idts)
                nc.vector.tensor_copy(
                    out=catT[7][:, qc * 128:(qc + 1) * 128],
                    in_=tpb[:, qc * 128:(qc + 1) * 128])

            for qc in range(NQC):
                pv_sweep(NH - 1, qc)
                if qc >= 1:
                    tail_tp(qc - 1)
                if qc >= 2:
                    outproj(qc - 2, woTs_h[0])
            tail_tp(7)
            outproj(6, woTs_h[0])
            outproj(7, woTs_h[0])

    nc.compile()
    return nc


def make_in_maps(x, wq2, wk2, wv2, wo2, bo2):
    """Per-core input dicts from full (already 2-D) fp32 arrays."""
    bf = ml_dtypes.bfloat16

    def wblk(w2):
        # [colblk 8, 128 dkpart, NDC, 128 col]
        return np.ascontiguousarray(
            w2.T.astype(bf).reshape(NDC, 128, 8, 128).transpose(2, 1, 0, 3))

    wqt = wblk(wq2)
    wkt = wblk(wk2)
    wvt = wblk(wv2)
    wot = np.ascontiguousarray(wo2.T.astype(bf).reshape(NDC, 128, D))
    bo3 = np.ascontiguousarray(bo2.reshape(1, D).astype(bf))
    idt = np.eye(128, dtype=bf)
    in_maps = []
    for c in range(8):
        b, h = c // 2, c % 2
        xr = x[b] if h == 0 else np.concatenate([x[b, TQ:], x[b, :TQ]], axis=0)
        # [tblk 4, 128 dkpart, NDC, 512 t]
        xtc = np.ascontiguousarray(
            xr.T.astype(bf).reshape(NDC, 128, 4, 512).transpose(2, 1, 0, 3))
        in_maps.append({"xt": xtc, "wqt": wqt, "wkt": wkt, "wvt": wvt,
                        "wot": wot, "bo": bo3, "idt": idt})
    return in_maps


_CACHE = {}


def kernel(x, Wq, Wk, Wv, Wo, bo):
    if "nc" not in _CACHE:
        _CACHE["nc"] = build_nc()
    nc = _CACHE["nc"]
    x = np.ascontiguousarray(x, dtype=np.float32)
    in_maps = make_in_maps(
        x, np.asarray(Wq).reshape(D, D).astype(np.float32),
        np.asarray(Wk).reshape(D, D).astype(np.float32),
        np.asarray(Wv).reshape(D, D).astype(np.float32),
        np.asarray(Wo).astype(np.float32),
        np.asarray(bo).astype(np.float32))
    res = run_bass_kernel_spmd(nc, in_maps, core_ids=list(range(8)))
    out = np.empty((4, T, D), dtype=np.float32)
    for c in range(8):
        b, h = c // 2, c % 2
        out[b, h * TQ:(h + 1) * TQ] = res.results[c]["y"]
    return out
